# revision 24
# baseline (speedup 1.0000x reference)
"""RNN-T JointNet kernel for 8 Trainium2 NeuronCores.

out[b,t,u,:] = gelu_tanh(enc[b,t]@We + dec[b,u]@Wd + b1) @ Wfc

Sharding: flatten (B=4, T=512) -> 2048 rows, 256 contiguous rows per core.
Core c handles batch b=c//2, time slice t0=(c%2)*256 .. +256.

Mixed precision: the fc matmul dominates (32768x512x512 per core) and fp32
matmuls run at 1/4 PE rate, so hact and Wfc are bf16 (1 cycle/row). The
prologue projections are also bf16; the broadcast add + gelu input stay
fp32. Output is stored bf16 (halves the 512 MiB HBM write) and upcast on
host. Norm rel err ~3.8e-3, well under the 2e-2 gate.

All inputs are pre-tiled on host into the exact (128, free) SBUF layouts so
every input DMA is a contiguous >=1KiB-per-partition-line copy (the
transposed loads otherwise emit 512B descriptors and stretch startup).

Per-core engine budget @64 groups of 2 u's (PE is the floor: 1024 bf16
matmuls x 512 moving rows = 218.5 us streaming at 2.4 GHz, measured 224 us
busy at ~98% issue efficiency; GPSIMD cannot touch PSUM, so DVE evacuates):
  PE    : 16 matmuls/group, hact (128x128) stationary, Wfc
          streams 512 -> psum (128t, 2x512v)                 (~224 us)
  GPSIMD: broadcast add tmp[h,(2u,t)] = peb[h,t] + pdb[h,u]
          for h-blocks 1..3 only                             (~184 us)
  ACT   : bias-fused gelu for h-block 0 (2 instrs) + one big
          gelu over h-blocks 1..3 -> hact bf16               (~167 us)
  DVE   : psum (128,1024) fp32 -> osb bf16, prologue evac    (~166 us)
  SP    : 2 output DMAs/group, 256 KiB each, 2 KiB/partition (~94 us)
Plus ~10 us of fixed NEFF preamble/epilogue -> ~244 us measured
(8.59 GMAC/core; baseline all-fp32 version of this kernel: 910 us).
"""

import sys

import numpy as np

sys.path.insert(0, "/opt/trn_rl_repo")

import ml_dtypes

import concourse.bacc as bacc
import concourse.bass as bass
import concourse.mybir as mybir
import concourse.tile as tile
from concourse.bass_utils import run_bass_kernel_spmd

B, T, U, D, H, V = 4, 512, 128, 256, 512, 512
NCORES = 8
TC = (B * T) // NCORES  # 256 t-rows per core
UB = 2  # u's per main-loop group
NG = U // UB

_PROGRAM = None
LAST_RESULT = None


def _build():
    global _PROGRAM
    if _PROGRAM is not None:
        return _PROGRAM

    f32 = mybir.dt.float32
    bf16 = mybir.dt.bfloat16
    # Bacc (not raw Bass): its compile() pipeline moves matmul waits onto
    # ldweights and splits >1-wait instructions via event semaphores —
    # walrus rejects matmuls carrying 2 sync waits otherwise.
    nc = bacc.Bacc("TRN2", target_bir_lowering=False)

    # All inputs pre-tiled host-side to partition-major (128, free) layouts.
    w1we_d = nc.declare_dram_parameter("w1we", (128, 2 * H), bf16, isOutput=False)
    w1wd_d = nc.declare_dram_parameter("w1wd", (128, 2 * H), bf16, isOutput=False)
    encT_d = nc.declare_dram_parameter("encT", (128, 2 * TC), bf16, isOutput=False)
    decT_d = nc.declare_dram_parameter("decT", (128, 2 * U), bf16, isOutput=False)
    b1_d = nc.declare_dram_parameter("b1", (128, 4), f32, isOutput=False)
    wfc_d = nc.declare_dram_parameter("Wfc", (128, 4 * V), bf16, isOutput=False)
    out_d = nc.declare_dram_parameter("out", (TC, U, V), bf16, isOutput=True)

    GELU = mybir.ActivationFunctionType.Gelu_apprx_tanh

    with tile.TileContext(nc) as tc:
        with (
            tc.tile_pool(name="const", bufs=1) as cpool,
            tc.tile_pool(name="tmps", bufs=3) as tpool,
            tc.tile_pool(name="hacts", bufs=3) as hpool,
            tc.tile_pool(name="outsb", bufs=6) as osb_pool,
            tc.tile_pool(name="pro_ps", bufs=2, space="PSUM") as pro_ps,
            tc.tile_pool(name="out_ps", bufs=2, space="PSUM") as out_ps_pool,
        ):
            # w1we col-block di*H+h = We[di*128+p, h]; w1wd likewise for Wd.
            w1we_sb = cpool.tile([128, 2 * H], bf16)
            w1wd_sb = cpool.tile([128, 2 * H], bf16)
            wfc_sb = cpool.tile([128, 4 * V], bf16)  # block ht = Wfc[ht*128:...]
            b1_sb = cpool.tile([128, 4], f32)  # col ht = b1[ht*128:(ht+1)*128]
            encT_sb = cpool.tile([128, 2 * TC], bf16)
            decT_sb = cpool.tile([128, 2 * U], bf16)
            peb_sb = cpool.tile([128, 4 * TC], f32)  # [ht*TC+t] = enc@We
            pd_sb = cpool.tile([128, 4 * U], f32)  # [ht*U+u] = dec@Wd + b1

            # Issue order = consumption order; wfc (needed ~7us in) last.
            nc.sync.dma_start(w1we_sb, w1we_d[:, :])
            nc.sync.dma_start(encT_sb, encT_d[:, :])
            nc.sync.dma_start(w1wd_sb, w1wd_d[:, :])
            nc.sync.dma_start(decT_sb, decT_d[:, :])
            # b1 from the ACT queue: warms its DGE path so the tail's
            # ACT-issued output DMA doesn't pay the cold-queue latency.
            nc.scalar.dma_start(b1_sb, b1_d[:, :])
            nc.sync.dma_start(wfc_sb, wfc_d[:, :])

            # Prologue: peb[h,t] = enc@We ; pd[h,u] = dec@Wd + b1.
            # PSUM evacuation on DVE (idle early) so ACT can start the first
            # groups' bias-fused gelus as soon as each (peb, pd) block lands.
            for ht in range(4):
                pe_ps = pro_ps.tile([128, TC], f32)
                for di in range(2):
                    nc.tensor.matmul(
                        pe_ps,
                        w1we_sb[:, di * H + ht * 128 : di * H + (ht + 1) * 128],
                        encT_sb[:, di * TC : (di + 1) * TC],
                        start=(di == 0),
                        stop=(di == 1),
                    )
                nc.vector.tensor_copy(peb_sb[:, ht * TC : (ht + 1) * TC], pe_ps)
                pd_ps = pro_ps.tile([128, U], f32)
                for di in range(2):
                    nc.tensor.matmul(
                        pd_ps,
                        w1wd_sb[:, di * H + ht * 128 : di * H + (ht + 1) * 128],
                        decT_sb[:, di * U : (di + 1) * U],
                        start=(di == 0),
                        stop=(di == 1),
                    )
                nc.vector.tensor_scalar_add(
                    pd_sb[:, ht * U : (ht + 1) * U],
                    pd_ps,
                    b1_sb[:, ht : ht + 1],
                )

            # Broadcast-add source APs for h-blocks 1..3, iteration order
            # (u, ht, t): peb u-dim stride 0; pd t-dim stride 0.
            peb_bc = (
                peb_sb[:, TC : 4 * TC]
                .rearrange("p (i t) -> p i t", i=3)
                .unsqueeze(1)
                .broadcast_to((128, UB, 3, TC))
            )
            pd_iu = pd_sb.rearrange("p (i u) -> p i u", i=4)

            # Main loop over groups of UB u's. hact is a separate tile per
            # u-half: Tile dep-tracking is whole-tile, so a shared tile
            # would make PE's ui0 matmuls wait on ui1's gelu writes too.
            for g in range(NG):
                u0 = g * UB
                hacts = [
                    hpool.tile([128, 4 * TC], bf16, tag=f"hact{ui}", name=f"hact{ui}")
                    for ui in range(UB)
                ]
                if g < 1:
                    # First group: all h-blocks via ACT bias-fused gelu —
                    # no GPSIMD add in the dependence chain, so PE's output
                    # matmuls start while the prologue is still draining.
                    for ui in range(UB):
                        for ht in range(4):
                            nc.scalar.activation(
                                hacts[ui][:, ht * TC : (ht + 1) * TC],
                                peb_sb[:, ht * TC : (ht + 1) * TC],
                                GELU,
                                bias=pd_sb[:, ht * U + u0 + ui : ht * U + u0 + ui + 1],
                            )
                else:
                    # h-blocks 1..3: GPSIMD broadcast add (both u's, one
                    # instr), then per-u gelu + bias-fused h-block 0 gelu.
                    tmp = tpool.tile([128, UB * 3 * TC], f32, tag="tmp")
                    pd_bc = (
                        pd_iu[:, 1:4, u0 : u0 + UB]
                        .transpose([0, 2, 1])
                        .unsqueeze(3)
                        .broadcast_to((128, UB, 3, TC))
                    )
                    nc.gpsimd.tensor_tensor(
                        tmp.rearrange("p (u i t) -> p u i t", u=UB, i=3),
                        peb_bc,
                        pd_bc,
                        mybir.AluOpType.add,
                    )
                    for ui in range(UB):
                        nc.scalar.activation(
                            hacts[ui][:, TC : 4 * TC],
                            tmp[:, ui * 3 * TC : (ui + 1) * 3 * TC],
                            GELU,
                        )
                        nc.scalar.activation(
                            hacts[ui][:, 0:TC],
                            peb_sb[:, 0:TC],
                            GELU,
                            bias=pd_sb[:, u0 + ui : u0 + ui + 1],
                        )

                # psum tile (128 t, 2 banks): [:, ui*512:+512] = out rows for
                # (t-block ts, u0+ui); contraction over 4 h-blocks. The final
                # group drains its two halves on separate engines/queues so
                # the tail after the last matmul is ~halved.
                last = g == NG - 1
                for ts in range(TC // 128):
                    ops = out_ps_pool.tile([128, UB * V], f32)
                    for ui in range(UB):
                        for ht in range(4):
                            nc.tensor.matmul(
                                ops[:, ui * V : (ui + 1) * V],
                                hacts[ui][
                                    :, ht * TC + ts * 128 : ht * TC + ts * 128 + 128
                                ],
                                wfc_sb[:, ht * V : (ht + 1) * V],
                                start=(ht == 0),
                                stop=(ht == 3),
                            )
                    if last:
                        # Drain each u-half on its own cast engine + DMA
                        # queue, with separate osb tiles (sharing one tile
                        # creates a whole-tile WAW dep that serializes).
                        osb0 = osb_pool.tile([128, V], bf16, name="osb0")
                        nc.vector.tensor_copy(osb0, ops[:, :V])
                        nc.sync.dma_start(
                            out_d[ts * 128 : (ts + 1) * 128, u0 : u0 + 1, :],
                            osb0[:, None, :],
                        )
                        osb1 = osb_pool.tile([128, V], bf16, name="osb1")
                        nc.scalar.copy(osb1, ops[:, V:])
                        nc.scalar.dma_start(
                            out_d[ts * 128 : (ts + 1) * 128, u0 + 1 : u0 + UB, :],
                            osb1[:, None, :],
                        )
                    else:
                        osb = osb_pool.tile([128, UB * V], bf16)
                        nc.vector.tensor_copy(osb, ops)
                        nc.sync.dma_start(
                            out_d[ts * 128 : (ts + 1) * 128, u0 : u0 + UB, :],
                            osb.rearrange("p (u v) -> p u v", u=UB),
                        )

    nc.compile()
    _PROGRAM = nc
    return nc


def kernel(enc, dec, W1, b1, Wfc):
    global LAST_RESULT
    nc = _build()
    bf = ml_dtypes.bfloat16
    enc = np.asarray(enc, dtype=np.float32)
    dec = np.asarray(dec, dtype=np.float32)
    W1 = np.asarray(W1, dtype=np.float32)
    b1 = np.asarray(b1, dtype=np.float32)
    Wfc = np.asarray(Wfc, dtype=np.float32)

    # Pre-tile to partition-major (128, free) SBUF layouts.
    def pmaj(x, nblk):  # (nblk*128, F) -> (128, nblk*F)
        F = x.shape[1]
        return np.ascontiguousarray(
            x.reshape(nblk, 128, F).transpose(1, 0, 2).reshape(128, nblk * F)
        )

    w1we = pmaj(W1[:D], 2).astype(bf)
    w1wd = pmaj(W1[D:], 2).astype(bf)
    wfct = pmaj(Wfc, 4).astype(bf)
    b1t = np.ascontiguousarray(b1.reshape(4, 128).T)

    in_maps = []
    for c in range(NCORES):
        b, t0 = c // 2, (c % 2) * TC
        in_maps.append(
            {
                "encT": pmaj(enc[b, t0 : t0 + TC, :].T, 2).astype(bf),
                "decT": pmaj(dec[b].T, 2).astype(bf),
                "w1we": w1we,
                "w1wd": w1wd,
                "b1": b1t,
                "Wfc": wfct,
            }
        )

    LAST_RESULT = run_bass_kernel_spmd(nc, in_maps, list(range(NCORES)))

    out = np.empty((B, T, U, V), np.float32)
    for c in range(NCORES):
        b, t0 = c // 2, (c % 2) * TC
        out[b, t0 : t0 + TC] = LAST_RESULT.results[c]["out"].astype(np.float32)
    return out
